# revision 57
# baseline (speedup 1.0000x reference)
"""Masked causal self-attention on 8 Trainium2 NeuronCores.

Sharding (Megatron-style): core c -> (batch b = c//4, head-group g = c%4).
Each core computes QKV projections for its 4 heads (512 of 2048 cols,
column-parallel), causal attention for those heads on its batch, and a
row-parallel slice of the output projection, producing a partial [S, D]
output. Host sums the 4 partials per batch and adds bp_eff = bp + bv@Wp
(the V bias commutes through softmax-normalized attention rows).

On-chip dataflow is fully transposed (feature-major) so no transposes are
ever needed:
  x^T (host-prepped)  --W as lhsT-->  Q^T, K^T [hd, S];  x^T as lhsT --> V [S, hd]
  S^T = (K^T tile).T @ Q^T            [Sk part, Sq free]
  attn^T = exp(S^T * scale) * tri     (no max subtraction: |scores| < ~1)
  rowsum = ones.T @ racc              (PE, M=1, f32r)
  O^T += (V tile).T @ attn^T          [hd part, Sq free]
  out_partial = (O^T tile).T @ Wp     [S part, D free]

Scheduling (vs the first working version, ~20%+ faster):
  - DMA streams (x-tile, wq-tile) pairs first (xt0 split in column
    quarters) so the first Q matmul starts ~2us in instead of ~30us.
  - QKV runs in 6 stages of 8 slot-tagged PSUM chains (kt-major issue) so
    PE streams at DMA rate during the upload and stage drains overlap
    (split ACT/DVE).
  - Causal staircase blocks are column-trimmed: block r only computes
    q-columns >= 128*r. The mask is applied on the PE itself: a -1e4
    identity-matmul seeds the scores PSUM group (full-width, start=True)
    and the exp then writes exact zeros -- no DVE/Pool op in the
    exp->O-matmul critical path. (GPSIMD measured ~3.2us/op on HW;
    banned from inner loops.)
  - Rowsum accumulates on DVE (f32r tiles), block 0 folded into block 1's
    add; partition-reduced by an f32r ones-matmul, broadcast back by a
    second one (no bf16 round-trips).
  - Attention chains run q-chunk (0,3,2,1) with per-chain tails deferred
    one chain; out_proj chains interleave into the stream 5-per-chain to
    fill PE gaps; out stores go out via DMA as bf16 partials.
  - Consecutive identical PE Ldweights are deduped post-emission.
"""

import os
import sys

import numpy as np

try:
    import concourse.bass as bass
except ImportError:
    sys.path.insert(0, "/opt/trn_rl_repo")
    import concourse.bass as bass

import ml_dtypes
import concourse.mybir as mybir
import concourse.tile as tile
from concourse.bass_utils import run_bass_kernel_spmd

BF16 = mybir.dt.bfloat16
F32 = mybir.dt.float32
F32R = mybir.dt.float32r
AF = mybir.ActivationFunctionType

B, S, D, H, HD = 2, 2048, 2048, 16, 128
NH = 4                # heads per core
HG = NH * HD          # 512: head-group width per core
NKT = D // 128        # 16 contraction k-tiles over D
NST = S // 128        # 16 s-tiles of 128
NQC = S // 512        # 4 q-chunks of 512
SCALE = 1.0 / float(np.sqrt(D))

# Bisection flags (env-settable) for HW-vs-sim divergence hunting.
# GPSIMD measured ~3.2us/op on HW (vs 0.4-1.1us modeled) -- keep it out of
# the attention inner loop.
USE_GPSIMD = os.environ.get("KOPT_GPSIMD", "0") == "1"
USE_F32R = os.environ.get("KOPT_F32R", "1") == "1"
USE_STAGED_QKV = os.environ.get("KOPT_STAGED_QKV", "1") == "1"
USE_INTERLEAVE = os.environ.get("KOPT_INTERLEAVE", "1") == "1"
USE_TRIM = os.environ.get("KOPT_TRIM", "1") == "1"
# Apply the causal mask as a PE accumulate of -1e4 into the scores PSUM
# (exp then yields exact zeros) instead of a DVE multiply after the exp.
USE_PE_MASK = os.environ.get("KOPT_PE_MASK", "1") == "1"
# Engine whose DMA queue carries the out stores ("sync"/"scalar"/"vector"/
# "gpsimd"): a non-SP queue frees the SP FIFO so the next loop iteration's
# input loads aren't queued behind this iteration's stores.
STORE_ENG = os.environ.get("KOPT_STORE_ENG", "sync")
USE_DEDUP_LDW = os.environ.get("KOPT_DEDUP_LDW", "1") == "1"
USE_WIDE_BUFS = os.environ.get("KOPT_WIDE_BUFS", "1") == "1"
WIDE_BUFS_N = int(os.environ.get("KOPT_WIDE_BUFS_N", "6"))
OP_POPS = int(os.environ.get("KOPT_OP_POPS", "5"))
# PSUM bank split between scores lookahead (ps_s) and out_proj accs (op):
# "3-2" or "2-3" (total is capped by the 8 banks with ps_o=2, ps_rb=1).
BANKSPLIT = os.environ.get("KOPT_BANKSPLIT", "2-3")

LAST_EXEC_NS = None


def dedup_ldweights(nc):
    """Drop PE Ldweights identical to the immediately preceding one (same
    weights AP, nothing reloaded in between): the PE weight register already
    holds that data. Sync info migrates onto the next instruction so all
    semaphore waits/updates still happen. Saves the reload cost that walrus
    (--enable-ldw-opt=false) won't elide."""
    def key(inst):
        a = inst.ins[0]
        return (a.memref, a.offset, str(a.ap), a.dtype,
                inst.perf_mode, inst.is_transpose, inst.tile_position)

    removed = 0
    for f in nc.m.functions:
        for bb in f.blocks:
            out = []
            last_ld = None
            pend_waits, pend_updates = [], []
            for inst in bb.instructions:
                if isinstance(inst, mybir.InstLdweights):
                    k = key(inst)
                    if last_ld is not None and k == last_ld:
                        si = inst.sync_info
                        if si is not None:
                            pend_waits.extend(si.on_wait)
                            pend_updates.extend(si.on_update)
                        removed += 1
                        continue
                    last_ld = k
                elif inst.engine == mybir.EngineType.PE and not isinstance(
                        inst, (mybir.InstMatmult, mybir.InstEventSemaphore)):
                    last_ld = None  # unknown PE instruction: be conservative
                if (pend_waits or pend_updates) and \
                        inst.engine == mybir.EngineType.PE:
                    si = inst.sync_info
                    if si is None:
                        inst.sync_info = mybir.SyncInfo(
                            on_wait=pend_waits, on_update=pend_updates)
                    else:
                        si.on_wait = pend_waits + list(si.on_wait)
                        si.on_update = list(si.on_update) + pend_updates
                    pend_waits, pend_updates = [], []
                out.append(inst)
            assert not pend_waits and not pend_updates
            bb.instructions = out
    return removed


def split_excess_waits(nc, maxw=1):
    """Walrus in this toolchain rejects >1 sync wait on CTRL-class
    instructions (Tile's tail drain can carry many). Hoist excess waits
    onto preceding single-wait EventSemaphore instructions."""
    for f in nc.m.functions:
        for bb in f.blocks:
            out, changed, k = [], False, 0
            for inst in bb.instructions:
                si = inst.sync_info
                if si is not None and len(si.on_wait) > maxw:
                    waits = list(si.on_wait)
                    while len(waits) > maxw:
                        chunk, waits = waits[:maxw], waits[maxw:]
                        out.append(mybir.InstEventSemaphore(
                            name=f"{inst.name}-waitsplit{k}", engine=inst.engine,
                            sync_info=mybir.SyncInfo(on_wait=chunk, on_update=[])))
                        k += 1
                        changed = True
                    si.on_wait = waits
                out.append(inst)
            if changed:
                bb.instructions = out


def qkv_proj(nc, tc, xT, wq, wk, wv, bqk_sb, QT, KT, V, const_dmas):
    """Phase 1: Q^T,K^T (feature-major) and V (token-major) projections.

    DMA order: (xt, wq) pairs kt-ascending (xt0 split into column quarters
    so the very first matmul starts ~1us in), consts after the 4th pair,
    then wk, then wv. Chains are issued kt-major in stages of 8 slot-tagged
    PSUM banks; the next stage's slot s only waits for slot s's drain, not
    the whole previous stage."""
    with tc.tile_pool(name="xw", bufs=1) as xw_pool, \
         tc.tile_pool(name="ps1", bufs=1, space="PSUM") as ps1:
        xt_t, w_t = [], {}
        # kt=0: x chunk 0, wq0, x chunks 1-3 -- first matmul fires off the
        # first ~1.3MB instead of the first 10MB.
        t0 = xw_pool.tile([128, S], BF16, tag="xt0", name="xt0")
        nc.sync.dma_start(t0[:, 0:512], xT[0:128, 0:512])
        tw0 = xw_pool.tile([128, HG], BF16, tag="wq0", name="wq0")
        nc.sync.dma_start(tw0[:], wq[0:128, :])
        for c in range(1, 4):
            nc.sync.dma_start(t0[:, c * 512:(c + 1) * 512],
                              xT[0:128, c * 512:(c + 1) * 512])
        xt_t.append(t0)
        w_t["q", 0] = tw0
        for kt in range(1, NKT):
            t = xw_pool.tile([128, S], BF16, tag=f"xt{kt}", name=f"xt{kt}")
            nc.sync.dma_start(t[:], xT[kt * 128:(kt + 1) * 128, :])
            xt_t.append(t)
            tw = xw_pool.tile([128, HG], BF16, tag=f"wq{kt}", name=f"wq{kt}")
            nc.sync.dma_start(tw[:], wq[kt * 128:(kt + 1) * 128, :])
            w_t["q", kt] = tw
            if kt == 3:
                for sb_tile, dram_t in const_dmas:
                    nc.sync.dma_start(sb_tile[:], dram_t[:])
        for nm, dram in (("k", wk), ("v", wv)):
            for kt in range(NKT):
                tw = xw_pool.tile([128, HG], BF16, tag=f"w{nm}{kt}",
                                  name=f"w{nm}{kt}")
                nc.sync.dma_start(tw[:], dram[kt * 128:(kt + 1) * 128, :])
                w_t[nm, kt] = tw

        if USE_STAGED_QKV:
            # Q^T and K^T: stages of m-pairs; within a stage, kt-major issue
            # over 8 chains (2 m x 4 nq). Drains alternate ACT/DVE so the
            # next stage un-stalls ~2x sooner.
            for nm, dstT, bcol in (("q", QT, 0), ("k", KT, NH)):
                for mp in (0, 2):
                    accs = {}
                    for m in (mp, mp + 1):
                        for nq in range(NQC):
                            slot = (m - mp) * NQC + nq
                            accs[m, nq] = ps1.tile(
                                [128, 512], F32, tag=f"s{slot}",
                                name=f"acc{slot}")
                    for kt in range(NKT):
                        for m in (mp, mp + 1):
                            for nq in range(NQC):
                                nc.tensor.matmul(
                                    accs[m, nq][:],
                                    w_t[nm, kt][:, m * 128:(m + 1) * 128],
                                    xt_t[kt][:, nq * 512:(nq + 1) * 512],
                                    start=(kt == 0), stop=(kt == NKT - 1),
                                )
                    for m in (mp, mp + 1):
                        for nq in range(NQC):
                            dst = dstT[:, m * S + nq * 512:
                                       m * S + nq * 512 + 512]
                            if nq % 2 == 0:
                                nc.scalar.activation(
                                    dst, accs[m, nq][:], AF.Identity,
                                    bias=bqk_sb[:, bcol + m: bcol + m + 1])
                            else:
                                nc.vector.tensor_scalar_add(
                                    dst, accs[m, nq][:],
                                    bqk_sb[:, bcol + m: bcol + m + 1])
            # V (token-major, no bias -- folded into bp on host): x^T tile
            # as lhsT, 2 stages of 8 s-tiles, kt-major.
            for sp in (0, 8):
                accs = [ps1.tile([128, HG], F32, tag=f"s{si}",
                                 name=f"vacc{si}") for si in range(8)]
                for kt in range(NKT):
                    for si in range(8):
                        st = sp + si
                        nc.tensor.matmul(
                            accs[si][:],
                            xt_t[kt][:, st * 128:(st + 1) * 128],
                            w_t["v", kt][:],
                            start=(kt == 0), stop=(kt == NKT - 1),
                        )
                for si in range(8):
                    st = sp + si
                    dst = V[:, st * HG:(st + 1) * HG]
                    if si % 2 == 0:
                        nc.scalar.copy(dst, accs[si][:])
                    else:
                        nc.vector.tensor_copy(dst, accs[si][:])
        else:
            # Baseline-style chain-major issue, groups of 4 over 8 banks.
            for nm, dstT, bcol in (("q", QT, 0), ("k", KT, NH)):
                for m in range(NH):
                    accs = [ps1.tile([128, 512], F32, tag="g", bufs=8,
                                     name=f"acc{i}") for i in range(NQC)]
                    for kt in range(NKT):
                        for nq in range(NQC):
                            nc.tensor.matmul(
                                accs[nq][:],
                                w_t[nm, kt][:, m * 128:(m + 1) * 128],
                                xt_t[kt][:, nq * 512:(nq + 1) * 512],
                                start=(kt == 0), stop=(kt == NKT - 1),
                            )
                    for nq in range(NQC):
                        nc.scalar.activation(
                            dstT[:, m * S + nq * 512: m * S + nq * 512 + 512],
                            accs[nq][:], AF.Identity,
                            bias=bqk_sb[:, bcol + m: bcol + m + 1],
                        )
            for sg in range(NST // 4):
                accs = [ps1.tile([128, HG], F32, tag="g", bufs=8,
                                 name=f"acc{i}") for i in range(4)]
                for kt in range(NKT):
                    for si in range(4):
                        st = 4 * sg + si
                        nc.tensor.matmul(
                            accs[si][:],
                            xt_t[kt][:, st * 128:(st + 1) * 128],
                            w_t["v", kt][:],
                            start=(kt == 0), stop=(kt == NKT - 1),
                        )
                for si in range(4):
                    st = 4 * sg + si
                    nc.vector.tensor_copy(V[:, st * HG:(st + 1) * HG],
                                          accs[si][:])


def emit_attn_outproj(nc, tc, QT, KT, V, OT, wp_t, tri_sb, ident_sb,
                      mneg_sb, onec_sb, oner_sb, out):
    """Phase 2+3: causal attention per head with out_proj chains interleaved.

    Loops are q-chunk-outer so a q-chunk's OT rows (all 4 heads) complete
    early; its out_proj chains are then slotted between the next q-chunk's
    head chains to fill PE gaps. Staircase block r only computes q-columns
    >= 128*r; the single [128,128] diagonal strip is masked on GPSIMD."""
    LOOK = 3
    wb = WIDE_BUFS_N if USE_WIDE_BUFS else 0
    with tc.tile_pool(name="attn", bufs=5 + wb) as attn_pool, \
         tc.tile_pool(name="fin", bufs=2 + (wb + 2) // 3) as fin_pool, \
         tc.tile_pool(name="outst", bufs=4 + wb) as outst, \
         tc.tile_pool(name="psA", bufs=1, space="PSUM") as psA:

        copy_tog = [0]

        def op_chain(ms, nc2):
            acc = psA.tile([128, 512], F32, tag="op",
                           bufs=int(BANKSPLIT.split("-")[1]), name="opacc")
            for hh in range(NH):
                nc.tensor.matmul(
                    acc[:],
                    OT[:, hh * S + ms * 128: hh * S + ms * 128 + 128],
                    wp_t[hh][:, nc2 * 512:(nc2 + 1) * 512],
                    start=(hh == 0), stop=(hh == NH - 1),
                )
            ot = outst.tile([128, 512], BF16, tag="outst", name="ot")
            # ACT drains out_proj (DVE is the attention-phase pole)
            nc.scalar.copy(ot[:], acc[:])
            copy_tog[0] += 1
            # out stores go out the Pool sequencer's DMA queue: frees the SP
            # FIFO so the next loop iteration's input loads aren't queued
            # behind this iteration's stores (cheap descriptor write, no Q7).
            store_eng = getattr(nc, STORE_ENG)
            store_eng.dma_start(
                out[ms * 128:(ms + 1) * 128, nc2 * 512:(nc2 + 1) * 512],
                ot[:])

        def attn_body(h, qc):
            hS = h * S
            q0 = qc * 512
            kt_lim = 4 * (qc + 1)
            acc_o = psA.tile([128, 512], F32, tag="ps_o", bufs=2, name="acc_o")
            # f32r so the PE rowsum/bcast matmuls run at bf16 rate without a
            # bf16 round-trip; every writer must round to f32r for walrus.
            racc = fin_pool.tile([128, 512], F32R if USE_F32R else F32,
                                 tag="racc", name="racc")

            def emit_scores(kt):
                r = kt - 4 * qc
                c0 = 128 * r if r > 0 else 0
                diag = r >= 0
                ps = psA.tile([128, 512], F32, tag="ps_s",
                              bufs=int(BANKSPLIT.split("-")[0]), name="ps")
                if diag and USE_PE_MASK:
                    # seed the PSUM with -1e4 on the diagonal strip (zeros
                    # right of it); the scores matmul accumulates on top and
                    # the exp then writes exact zeros in the masked strip.
                    nc.tensor.matmul(
                        ps[:, c0:512], ident_sb[:], mneg_sb[:, 0:512 - c0],
                        start=True, stop=False,
                    )
                nc.tensor.matmul(
                    ps[:, c0:512],
                    KT[:, hS + kt * 128: hS + kt * 128 + 128],
                    QT[:, hS + q0 + c0: hS + q0 + 512],
                    start=not (diag and USE_PE_MASK), stop=True,
                )
                at = attn_pool.tile([128, 512], BF16, tag="at", name="at")
                nc.scalar.activation(at[:, c0:512], ps[:, c0:512], AF.Exp,
                                     scale=SCALE)
                if diag and not USE_PE_MASK:
                    eng = nc.gpsimd if USE_GPSIMD else nc.vector
                    eng.tensor_mul(
                        at[:, c0:c0 + 128], at[:, c0:c0 + 128], tri_sb[:])
                return at, c0

            ats = {kt: emit_scores(kt) for kt in range(min(LOOK, kt_lim))}
            for kt in range(kt_lim):
                if kt + LOOK < kt_lim:
                    ats[kt + LOOK] = emit_scores(kt + LOOK)
                at, c0 = ats.pop(kt)
                if kt == 0:
                    at_prev = at  # folded into kt=1's add (saves a 512 copy)
                elif kt == 1:
                    if c0 > 0:
                        nc.vector.tensor_copy(racc[:, 0:c0],
                                              at_prev[:, 0:c0])
                    nc.vector.tensor_add(racc[:, c0:512], at_prev[:, c0:512],
                                         at[:, c0:512])
                else:
                    nc.vector.tensor_add(racc[:, c0:512], racc[:, c0:512],
                                         at[:, c0:512])
                nc.tensor.matmul(
                    acc_o[:, c0:512],
                    V[:, kt * HG + h * 128: kt * HG + h * 128 + 128],
                    at[:, c0:512],
                    start=(kt == 0), stop=(kt == kt_lim - 1))
            return (h, qc, acc_o, racc)

        def attn_tail(h, qc, acc_o, racc):
            hS = h * S
            q0 = qc * 512
            # normalize: O^T[:, i] /= rowsum[i] (f32r keeps fp32 PE rate)
            acc_r = psA.tile([1, 512], F32, tag="ps_rb", bufs=1,
                             name="acc_r")
            if USE_F32R:
                rsum_rhs = racc[:]
            else:
                rb = fin_pool.tile([128, 512], BF16, tag="rb", name="rb")
                nc.vector.tensor_copy(rb[:], racc[:])
                rsum_rhs = rb[:]
            nc.tensor.matmul(acc_r[:], onec_sb[:], rsum_rhs,
                             start=True, stop=True)
            rs = fin_pool.tile([1, 512], F32R if USE_F32R else BF16,
                               tag="rs", name="rs")
            with nc.allow_low_precision(reason="rowsum reciprocal fmt"):
                nc.vector.reciprocal(rs[:], acc_r[:])
            bc = psA.tile([128, 512], F32, tag="ps_rb", bufs=1, name="bc")
            nc.tensor.matmul(bc[:], oner_sb[:], rs[:],
                             start=True, stop=True)
            rcp = fin_pool.tile([128, 512], F32, tag="rcp", name="rcp")
            nc.vector.tensor_copy(rcp[:], bc[:])
            nc.vector.tensor_mul(
                OT[:, hS + q0: hS + q0 + 512], acc_o[:], rcp[:])

        # q-chunks descending (longest chains first: best PE pipelining at
        # phase entry). Chain tails are deferred one chain so their serial
        # rowsum->recip->bcast->norm latency hides under the next body; a
        # q-chunk's out_proj chains become poppable only after its last tail
        # plus one body of slack.
        # qc=0 first seeds the out_proj pop queue cheaply, then longest
        # chains descending while pops fill PE gaps.
        chains = [(h, qc) for qc in (0, 3, 2, 1) for h in range(NH)]
        pending, prev, prev_done_qc = [], None, None
        for h, qc in chains:
            st = attn_body(h, qc)
            if prev is not None:
                attn_tail(*prev)
                if USE_INTERLEAVE:
                    for _ in range(OP_POPS):
                        if pending:
                            op_chain(*pending.pop(0))
                if prev[0] == NH - 1:  # finished a q-chunk's last tail
                    pending += [(ms, nc2)
                                for ms in range(4 * prev[1], 4 * prev[1] + 4)
                                for nc2 in range(NQC)]
            prev = st
        attn_tail(*prev)
        pending += [(ms, nc2) for ms in range(4 * prev[1], 4 * prev[1] + 4)
                    for nc2 in range(NQC)]
        for ms, nc2 in pending:
            op_chain(ms, nc2)


def emit_all(nc, tc, xT, wq, wk, wv, wp, out, bqk_sb, tri_sb, ident_sb,
             mneg_sb, onec_sb, oner_sb, QT, KT, V, OT, const_dmas):
    qkv_proj(nc, tc, xT, wq, wk, wv, bqk_sb, QT, KT, V, const_dmas)
    with tc.tile_pool(name="wp_pool", bufs=1) as wp_pool:
        wp_t = []
        for h in range(NH):
            t = wp_pool.tile([128, D], BF16, tag=f"wp{h}", name=f"wp{h}")
            nc.sync.dma_start(t[:], wp[h * 128:(h + 1) * 128, :])
            wp_t.append(t)
        emit_attn_outproj(nc, tc, QT, KT, V, OT, wp_t, tri_sb, ident_sb,
                          mneg_sb, onec_sb, oner_sb, out)


def build(loop_n=1):
    nc = bass.Bass()

    xT = nc.declare_dram_parameter("xT", [D, S], BF16, isOutput=False)
    wq = nc.declare_dram_parameter("wq", [D, HG], BF16, isOutput=False)
    wk = nc.declare_dram_parameter("wk", [D, HG], BF16, isOutput=False)
    wv = nc.declare_dram_parameter("wv", [D, HG], BF16, isOutput=False)
    wp = nc.declare_dram_parameter("wp", [HG, D], BF16, isOutput=False)
    bqk = nc.declare_dram_parameter("bqk", [128, 2 * NH], F32, isOutput=False)
    tri = nc.declare_dram_parameter("tri", [128, 128], BF16, isOutput=False)
    identm = nc.declare_dram_parameter("identm", [128, 128], BF16, isOutput=False)
    mneg = nc.declare_dram_parameter("mneg", [128, 512], BF16, isOutput=False)
    ODT = F32R if USE_F32R else BF16
    ones_col = nc.declare_dram_parameter("ones_col", [128, 1], ODT, isOutput=False)
    ones_row = nc.declare_dram_parameter("ones_row", [1, 128], ODT, isOutput=False)
    out = nc.declare_dram_parameter("out", [S, D], BF16, isOutput=True)

    with tile.TileContext(nc) as tc:
        with tc.tile_pool(name="const", bufs=1) as cpool, \
             tc.tile_pool(name="qkv", bufs=1) as qkv_pool:
            # const DMAs are deferred into qkv_proj's stream (after the 4th
            # x/w pair) so they don't delay the first matmul's inputs.
            bqk_sb = cpool.tile([128, 2 * NH], F32, tag="bqk")
            tri_sb = cpool.tile([128, 128], BF16, tag="tri")
            ident_sb = cpool.tile([128, 128], BF16, tag="identm")
            mneg_sb = cpool.tile([128, 512], BF16, tag="mneg")
            onec_sb = cpool.tile([128, 1], ODT, tag="onec")
            oner_sb = cpool.tile([1, 128], ODT, tag="oner")
            const_dmas = [(bqk_sb, bqk), (onec_sb, ones_col),
                          (oner_sb, ones_row)]
            if USE_PE_MASK:
                const_dmas += [(ident_sb, identm), (mneg_sb, mneg)]
            else:
                const_dmas += [(tri_sb, tri)]

            # Per-head feature-major Q^T/K^T/O^T: head h lives in cols
            # [h*S, (h+1)*S). V is token-major: s-tile st in cols
            # [st*HG, (st+1)*HG).
            QT = qkv_pool.tile([128, NH * S], BF16, tag="QT")
            KT = qkv_pool.tile([128, NH * S], BF16, tag="KT")
            V = qkv_pool.tile([128, NST * HG], BF16, tag="V")
            OT = qkv_pool.tile([128, NH * S], BF16, tag="OT")

            if loop_n == 1:
                emit_all(nc, tc, xT, wq, wk, wv, wp, out, bqk_sb, tri_sb,
                         ident_sb, mneg_sb, onec_sb, oner_sb, QT, KT, V, OT,
                         const_dmas)
            else:
                with tc.For_i(0, loop_n, 1) as _i:
                    emit_all(nc, tc, xT, wq, wk, wv, wp, out, bqk_sb, tri_sb,
                             ident_sb, mneg_sb, onec_sb, oner_sb, QT, KT, V,
                             OT, const_dmas)
    if USE_DEDUP_LDW:
        dedup_ldweights(nc)
    split_excess_waits(nc)
    return nc


_NC_CACHE = {}


def _get_nc(loop_n=1):
    if loop_n not in _NC_CACHE:
        _NC_CACHE[loop_n] = build(loop_n)
    return _NC_CACHE[loop_n]


def _prep_in_maps(x, Wq, bq, Wk, bk, Wv, bv, Wp, bp):
    x = np.asarray(x, dtype=np.float32)
    bf = ml_dtypes.bfloat16
    jj = np.arange(128)[:, None]
    ii = np.arange(128)[None, :]
    tri = (ii >= jj).astype(np.float32).astype(bf)
    identm = np.eye(128, dtype=np.float32).astype(bf)
    mneg = np.concatenate([np.where(ii >= jj, 0.0, -1e4),
                           np.zeros((128, 384))], axis=1
                          ).astype(np.float32).astype(bf)
    odt = np.float32 if USE_F32R else bf
    ones_col = np.ones((128, 1), dtype=odt)
    ones_row = np.ones((1, 128), dtype=odt)

    xTb = [np.ascontiguousarray(x[b].T).astype(bf) for b in range(B)]
    in_maps = []
    for c in range(8):
        b, g = divmod(c, 4)
        sl = slice(g * HG, (g + 1) * HG)
        bqk = np.concatenate(
            [np.asarray(bq)[sl].reshape(NH, 128).T,
             np.asarray(bk)[sl].reshape(NH, 128).T], axis=1
        ).astype(np.float32)
        in_maps.append({
            "xT": xTb[b],
            "wq": np.ascontiguousarray(np.asarray(Wq)[:, sl]).astype(bf),
            "wk": np.ascontiguousarray(np.asarray(Wk)[:, sl]).astype(bf),
            "wv": np.ascontiguousarray(np.asarray(Wv)[:, sl]).astype(bf),
            "wp": np.ascontiguousarray(np.asarray(Wp)[sl, :]).astype(bf),
            "bqk": bqk,
            "tri": tri,
            "identm": identm,
            "mneg": mneg,
            "ones_col": ones_col,
            "ones_row": ones_row,
        })
    return in_maps


def kernel(x, Wq, bq, Wk, bk, Wv, bv, Wp, bp):
    global LAST_EXEC_NS
    # NTFF tracing needs antenv.axon_hooks, absent in this container; a set
    # BASS_TRACE would crash run_bass_kernel_spmd otherwise.
    os.environ["BASS_NEVER_TRACE"] = "1"
    nc = _get_nc()
    in_maps = _prep_in_maps(x, Wq, bq, Wk, bk, Wv, bv, Wp, bp)
    res = run_bass_kernel_spmd(nc, in_maps, core_ids=list(range(8)))
    LAST_EXEC_NS = res.exec_time_ns
    # bv commutes through the normalized attention rows: each partial's
    # missing contribution is bv[slice] @ Wp[slice, :]; summed over the 4
    # head-group cores that is bv @ Wp, folded into the bias here.
    bp_eff = (np.asarray(bp, dtype=np.float32)
              + np.asarray(bv, dtype=np.float32)
              @ np.asarray(Wp, dtype=np.float32))
    out = np.empty((B, S, D), dtype=np.float32)
    for b in range(B):
        acc = res.results[4 * b]["out"].astype(np.float32)
        for g in range(1, 4):
            acc = acc + res.results[4 * b + g]["out"].astype(np.float32)
        out[b] = acc
    out += bp_eff[None, None, :]
    return out


def _make_runner(nc, in_maps):
    """Replicate bass2jax.run_bass_via_pjrt's shard_map jit, returning a
    zero-arg callable over device-resident inputs (for repeat timing)."""
    import jax
    from jax.sharding import Mesh, PartitionSpec, NamedSharding
    from jax.experimental.shard_map import shard_map
    from concourse import bass2jax, mybir as _mybir
    from concourse.bass2jax import _bass_exec_p, install_neuronx_cc_hook

    install_neuronx_cc_hook()
    n_cores = len(in_maps)
    partition_name = (nc.partition_id_tensor.name
                      if nc.partition_id_tensor else None)
    in_names, out_names, out_avals, zero_outs = [], [], [], []
    for alloc in nc.m.functions[0].allocations:
        if not isinstance(alloc, _mybir.MemoryLocationSet):
            continue
        name = alloc.memorylocations[0].name
        if alloc.kind == "ExternalInput":
            if name != partition_name:
                in_names.append(name)
        elif alloc.kind == "ExternalOutput":
            out_names.append(name)
            shape = tuple(alloc.tensor_shape)
            dtype = _mybir.dt.np(alloc.dtype)
            out_avals.append(jax.core.ShapedArray(shape, dtype))
            zero_outs.append(np.zeros(shape, dtype))
    n_params = len(in_names)
    n_outs = len(out_avals)
    in_names = in_names + out_names
    if partition_name is not None:
        in_names.append(partition_name)

    def _body(*args):
        operands = list(args)
        if partition_name is not None:
            operands.append(bass2jax.partition_id_tensor())
        outs = _bass_exec_p.bind(
            *operands, out_avals=tuple(out_avals), in_names=tuple(in_names),
            out_names=tuple(out_names), lowering_input_output_aliases=(),
            sim_require_finite=True, sim_require_nnan=True, nc=nc)
        return tuple(outs)

    devices = jax.devices()[:n_cores]
    mesh = Mesh(np.asarray(devices), ("core",))
    in_specs = (PartitionSpec("core"),) * (n_params + n_outs)
    out_specs = (PartitionSpec("core"),) * len(out_names)
    fn = jax.jit(
        shard_map(_body, mesh=mesh, in_specs=in_specs, out_specs=out_specs,
                  check_rep=False),
        keep_unused=True)
    sh = NamedSharding(mesh, PartitionSpec("core"))
    concat_in = [
        jax.device_put(
            np.concatenate([np.asarray(in_maps[c][in_names[i]])
                            for c in range(n_cores)], axis=0), sh)
        for i in range(n_params)
    ]
    concat_zeros = [
        jax.device_put(np.zeros((n_cores * z.shape[0], *z.shape[1:]), z.dtype), sh)
        for z in zero_outs
    ]
    args = concat_in + concat_zeros

    def run():
        return fn(*args)

    return run


def _time_runner(run, iters):
    import time
    import jax
    jax.block_until_ready(run())  # compile + warm
    times = []
    for _ in range(iters):
        t0 = time.perf_counter()
        jax.block_until_ready(run())
        times.append(time.perf_counter() - t0)
    times.sort()
    return times


def benchmark(inputs, iters=3, loop_n=256, rounds=4):
    """Estimate per-execution HW time by amplifying the kernel body with an
    on-device For_i loop. Wall clocks quantize to ~9-10 ms tunnel poll
    boundaries, so use a large loop_n (quantization becomes +-35us/call)
    and average interleaved rounds."""
    import time
    import jax
    in_maps = _prep_in_maps(**inputs)
    run1 = _make_runner(_get_nc(1), in_maps)
    runN = _make_runner(_get_nc(loop_n), in_maps)
    jax.block_until_ready(run1())  # compile + warm
    jax.block_until_ready(runN())
    t1s, tNs = [], []
    for r in range(rounds):
        for _ in range(iters):
            t0 = time.perf_counter()
            jax.block_until_ready(run1())
            t1s.append(time.perf_counter() - t0)
            t0 = time.perf_counter()
            jax.block_until_ready(runN())
            tNs.append(time.perf_counter() - t0)
        print(f"  round {r}: wall(1) {t1s[-1]*1e3:.1f} ms "
              f"wall({loop_n}) {tNs[-1]*1e3:.1f} ms")
    # median, not mean: rare ~300ms RPC hiccups poison a mean while the
    # ~9ms poll quantization stays zero-median over enough samples.
    m1 = float(np.median(t1s))
    mN = float(np.median(tNs))
    est = (mN - m1) / (loop_n - 1)
    print(f"benchmark: med wall(1) {m1*1e3:.2f} ms, med wall({loop_n}) "
          f"{mN*1e3:.2f} ms -> est {est*1e6:.0f} us/exec")
    return est * 1e9


def benchmark3(inputs, iters=12, points=(1, 8, 32)):
    """Least-squares slope over several loop_n points; robust against
    per-executable constant offsets that contaminate the 2-point diff."""
    in_maps = _prep_in_maps(**inputs)
    xs, ys = [], []
    for n in points:
        run = _make_runner(_get_nc(n), in_maps)
        t = _time_runner(run, iters)
        med = t[len(t) // 2]
        print(f"  wall({n}) med {med*1e3:.2f} ms")
        xs.append(n)
        ys.append(med)
    xs = np.asarray(xs, dtype=np.float64)
    ys = np.asarray(ys, dtype=np.float64)
    slope = ((xs - xs.mean()) * (ys - ys.mean())).sum() / \
        ((xs - xs.mean()) ** 2).sum()
    print(f"benchmark3: slope {slope*1e6:.0f} us/exec over {list(points)}")
    return slope * 1e9
